# revision 46
# baseline (speedup 1.0000x reference)
"""Cross-attention kernel for Trainium2 (8 NeuronCores, Bass/Tile).

Problem: nn_CrossAttention — B=4, C=256, H=W=64 (N=4096 tokens), CI=128.
  q = q_w @ x + q_b            [B, N, CI]
  k = k_w @ rgbd + k_b         [B, CI, N]
  v = v_w @ rgbd + v_b         [B, N, CI]
  out = rgbd + out_w @ (softmax(q k) v) + out_b

Sharding: data-parallel over batch x query-half. Core i handles batch i//2,
query half i%2 (2048 queries, all 4096 keys). No collectives needed.

Math simplifications (exact):
  - k_b drops out of softmax (adds a per-query constant to logits).
  - v_b commutes with the softmax average; out_b + out_w @ v_b is folded
    into the residual on the host (res' = rgbd_slice + fused bias).
  - exp() without max-subtraction: logits are bounded (|S| <~ 45), safe fp32.

Engine plan (per core) — PE is the wall (~86us of matmul streaming), so the
kernel is one long software pipeline in which the in-order PE queue never
waits on anything:
  - One packed weights DMA (HWDGE dispatches cost ~650ns each on the serial
    Sync queue, and only ~8 transfers fit the DMA ring window); rs-heavy
    dispatch interleave since K/V^T consume rs long before QT needs xs.
  - Dep-free warmup matmuls bridge the input-DMA wait so the PE HAM
    clock-gate is at 2.4GHz when real work lands; an "ignition" first
    S-batch runs in 256-query halves off ~128KB first transfers.
  - V^T[k, ci] is computed directly as 64 small matmuls with rs chunks as
    the stationary operand (no PE transposes; ~56ns/MM issue cadence).
  - exp is split across engines: ScalarE runs true EXP ACTIVATEs; DVE runs
    Schraudolph bit-trick exp (x*128*log2e + 128*127-5 -> int16, bitcast
    bf16, ~3% rel err) for batches b3 (+b2 in tail-free steps).  Neither
    engine saturates, and the 6-bank S-ring (3 tiles x [128,2,512]) plus
    J-2 consumption lag keep every PE operand ready a quarter in advance:
    steady-state periods are set purely by PE matmul throughput.
  - Denominator rows accumulate via 4-way column-packed M=32 ones-matmuls
    (4 concurrent col-group streams); a bf16 ones/32 matmul folds them to
    d broadcast across partitions.
  - 1/d via DVE reciprocal_approx_fast (no ScalarE Ln -> exactly one ACT
    table load for the whole kernel).  O leaves PSUM unnormalized at chunk
    close (frees the single accumulator bank early); the out-projection
    runs on unnormalized O and 1/d folds into the final per-half multiply.
  - Tails (fold, rec, out-proj, normalize+residual, per-half store) run in
    stages inside later steps' slack; the last step catches up the J-1
    quarter after its final S batch so the epilogue is one quarter deep.
"""

import numpy as np

B, C, HH, WW = 4, 256, 64, 64
CI = 128
N = HH * WW            # 4096 tokens per batch
NCORES = 8
QSH = N // 2           # 2048 queries per core
QCH = 512              # query chunk (matmul moving free dim)
NQC = QSH // QCH       # 4 q-chunks
NKC = N // 128         # 32 key chunks of 128
EB = 2                 # S tiles per exp batch (PSUM-sourced ACTIVATE)
KQ = 8                 # key chunks per ET quarter-buffer
NQUARTER = NKC // KQ   # 4 quarters

# packed-weights byte offsets (per partition)
W_QW, W_KW, W_VWT, W_OW, W_QB = 0, 512, 1024, 1536, 2048
W_BYTES = 2052

_CACHE = {}


def _build_nc():
    import concourse.bass as bass
    import concourse.mybir as mybir
    import concourse.tile as tile
    from concourse import bacc
    from concourse.bass import ts

    f32 = mybir.dt.float32
    f32r = mybir.dt.float32r
    bf16 = mybir.dt.bfloat16
    u8 = mybir.dt.uint8
    EXP = mybir.ActivationFunctionType.Exp

    nc = bacc.Bacc("TRN2", target_bir_lowering=False, debug=False)

    rs_d = nc.dram_tensor("rs", [C, N], bf16, kind="ExternalInput")
    xs_d = nc.dram_tensor("xs", [C, QSH], bf16, kind="ExternalInput")
    res_d = nc.dram_tensor("res", [C, QSH], bf16, kind="ExternalInput")
    wp_d = nc.dram_tensor("wp", [128, W_BYTES], u8, kind="ExternalInput")
    out_d = nc.dram_tensor("out", [C, QSH], f32, kind="ExternalOutput")

    rs_r = rs_d.ap().rearrange("(co ci) n -> ci co n", ci=128)
    xs_r = xs_d.ap().rearrange("(co ci) n -> ci co n", ci=128)
    res_r = res_d.ap().rearrange("(co ci) n -> ci co n", ci=128)
    out_r = out_d.ap().rearrange("(co ci) n -> ci co n", ci=128)

    with tile.TileContext(nc) as tc:
        with (
            tc.tile_pool(name="const", bufs=1) as cpool,
            tc.tile_pool(name="big", bufs=3) as bigpool,
            tc.tile_pool(name="work", bufs=2) as wpool,
            tc.tile_pool(name="ps_s", bufs=3, space=bass.MemorySpace.PSUM) as ps_s,
            tc.tile_pool(name="ps_d", bufs=1, space=bass.MemorySpace.PSUM) as ps_d,
            tc.tile_pool(name="ps_o", bufs=1, space=bass.MemorySpace.PSUM) as ps_o,
        ):
            # ---- one packed weight DMA, then activations (xs before rs) ----
            wp_sb = cpool.tile([128, W_BYTES], u8, tag="wp")
            nc.sync.dma_start(wp_sb[:], wp_d.ap())

            rs_sb = cpool.tile([128, 2, N], bf16, tag="rs")
            xs_sb = cpool.tile([128, 2, QSH], bf16, tag="xs")
            res_sb = cpool.tile([128, 2, QSH], bf16, tag="res")
            # small first chunks so the first S batch (and its exp) ignites
            # on ~128KB transfers; then rs-heavy interleave -- the K/V^T
            # projections consume rs chunks much earlier than QT needs xs,
            # and only ~8 transfers fit in the DMA ring window at once
            nc.sync.dma_start(rs_sb[:, :, 0:256], rs_r[:, :, 0:256])
            nc.sync.dma_start(xs_sb[:, :, 0:256], xs_r[:, :, 0:256])
            nc.sync.dma_start(xs_sb[:, :, 256:512], xs_r[:, :, 256:512])
            nc.sync.dma_start(rs_sb[:, :, 256:512], rs_r[:, :, 256:512])
            xs_order = [None, 1, None, 2, None, None, 3, None]
            for j in range(1, 8):
                nc.sync.dma_start(rs_sb[:, :, ts(j, 512)], rs_r[:, :, ts(j, 512)])
                xj = xs_order[j]
                if xj is not None:
                    nc.sync.dma_start(xs_sb[:, :, ts(xj, 512)],
                                      xs_r[:, :, ts(xj, 512)])
            for j in range(NQC):
                nc.sync.dma_start(res_sb[:, :, ts(j, QCH)], res_r[:, :, ts(j, QCH)])

            qw_sb = wp_sb[:, W_QW:W_QW + 512].bitcast(bf16).rearrange(
                "p (co m) -> p co m", co=2)
            kw_sb = wp_sb[:, W_KW:W_KW + 512].bitcast(bf16).rearrange(
                "p (co m) -> p co m", co=2)
            vwt_sb = wp_sb[:, W_VWT:W_VWT + 512].bitcast(bf16).rearrange(
                "p (co m) -> p co m", co=2)
            ow_sb = wp_sb[:, W_OW:W_OW + 512].bitcast(bf16).rearrange(
                "p (t m) -> p t m", t=2)
            qb_sb = wp_sb[:, W_QB:W_QB + 4].bitcast(f32)

            # ---- constants (wsrc first: it gates the warmup matmuls) ----
            wsrc = cpool.tile([128, 512], bf16, tag="wsrc")
            nc.vector.memset(wsrc[:], 1.0)
            ones_f = cpool.tile([128, 128], f32, tag="ones_f")
            nc.vector.memset(ones_f[:], 1.0)
            ones32 = cpool.tile([128, 32], bf16, tag="ones32")
            nc.vector.tensor_copy(ones32[:], ones_f[:, :32])
            onesR = cpool.tile([128, 128], bf16, tag="onesR")
            nc.vector.memset(onesR[:], 1.0 / 32.0)

            # warm the ScalarE exp table while input DMAs run
            warm = cpool.tile([128, 1], f32, tag="warm")
            nc.scalar.activation(warm[:], ones_f[:, :1], EXP)

            # dep-free dummy matmuls: trip the PE HAM activity window during
            # the input-DMA wait so real work runs at full clock (they live
            # in the ps_s ring)
            for w in range(3):
                wps = ps_s.tile([128, EB, QCH], f32, tag="sps", name="wps")
                for i in range(EB):
                    nc.tensor.matmul(wps[:, i, :], wsrc[:, :128], wsrc[:])

            K_sb = cpool.tile([128, N], bf16, tag="K")
            QT_sb = cpool.tile([128, QSH], bf16, tag="QT")
            V_sb = cpool.tile([128, N], bf16, tag="V")

            # ---- prologue PE jobs (emitted into early pipeline slack;
            # transient psum comes from the S ring -- bufs=3 absorbs it) ----
            def qt_proj(p, halves=(slice(0, 512),)):
                rt = ps_s.tile([128, EB, QCH], f32, tag="sps", name="qtps")
                ps = rt[:, 0, :]
                base = 512 * p
                for h in halves:
                    for co in range(2):
                        nc.tensor.matmul(
                            ps[:, h], qw_sb[:, co, :],
                            xs_sb[:, co, base + h.start:base + h.stop],
                            start=(co == 0), stop=(co == 1))
                    nc.vector.tensor_scalar_add(
                        QT_sb[:, base + h.start:base + h.stop], ps[:, h], qb_sb)

            def k_proj(p, halves=(slice(0, 512),)):
                rt = ps_s.tile([128, EB, QCH], f32, tag="sps", name="kps")
                ps = rt[:, 0, :]
                base = 512 * p
                for h in halves:
                    for co in range(2):
                        nc.tensor.matmul(
                            ps[:, h], kw_sb[:, co, :],
                            rs_sb[:, co, base + h.start:base + h.stop],
                            start=(co == 0), stop=(co == 1))
                    nc.vector.tensor_copy(
                        K_sb[:, base + h.start:base + h.stop], ps[:, h])

            def vt_group(g):
                # V^T[k, ci] for key chunks 4g..4g+3: rs chunk stationary,
                # v_wT moving -> output keys on partitions (no transposes)
                rt = ps_s.tile([128, EB, QCH], f32, tag="sps", name="vps")
                ps = rt[:, 0, :]
                for j in range(4):
                    kc = 4 * g + j
                    for co in range(2):
                        nc.tensor.matmul(
                            ps[:, ts(j, 128)],
                            rs_sb[:, co, ts(kc, 128)],
                            vwt_sb[:, co, :],
                            start=(co == 0), stop=(co == 1))
                nc.vector.tensor_copy(V_sb[:, ts(g, 512)], ps)

            # extras[j] = prologue jobs to interleave into quarter-step j
            extras = {
                0: [lambda: k_proj(1), lambda: vt_group(0), lambda: k_proj(2),
                    lambda: vt_group(1), lambda: k_proj(3)],
                1: [lambda: k_proj(4), lambda: vt_group(2), lambda: vt_group(3)],
                2: [lambda: k_proj(5), lambda: vt_group(4), lambda: k_proj(6),
                    lambda: vt_group(5)],
                3: [lambda: k_proj(7), lambda: vt_group(6), lambda: vt_group(7),
                    lambda: qt_proj(1)],
                6: [lambda: qt_proj(2)],
                10: [lambda: qt_proj(3)],
            }

            # ---- pipeline state ----
            state = {}     # qc -> [dps, ops] PSUM accumulators
            ets = {}       # (qc,qq) -> et tile
            tail1 = {}     # qc -> (d_part, rec, o_sc) after stage 1

            # exp(x) ~= bitcast_bf16(int16(x*128*log2(e) + 128*127 - c)):
            # Schraudolph's approximation (~3% rel err; fine at 2e-2 tol).
            # Batches b2/b3 of each quarter run on DVE this way: the psum
            # ring slots that gate the next quarter's first S matmuls are
            # then freed by the (early, slack) DVE queue instead of by
            # ScalarE's last in-order exp -- PE never stalls on the ring.
            SCH_A = 128.0 / float(np.log(2.0))
            SCH_B = 128.0 * 127.0 - 5.0
            i16 = mybir.dt.int16
            MUL, ADD = mybir.AluOpType.mult, mybir.AluOpType.add

            def s_batch(qc, qq, b):
                # two S matmuls + one EXP covering key chunks (2b, 2b+1)
                qsl = ts(qc, QCH)
                if (qc, qq) not in ets:
                    ets[(qc, qq)] = bigpool.tile([128, KQ, QCH], bf16, tag="big", name="et")
                et = ets[(qc, qq)]
                sps = ps_s.tile([128, EB, QCH], f32, tag="sps", name="sps")
                for i in range(EB):
                    kc = qq * KQ + b * EB + i
                    nc.tensor.matmul(sps[:, i, :], K_sb[:, ts(kc, 128)],
                                     QT_sb[:, qsl])
                # DVE takes b3 (and b2 in steps without tail DVE work) so
                # neither exp engine saturates.
                on_dve = b == 3 or (b == 2 and qq in (0, 1))
                if on_dve:
                    nc.vector.tensor_scalar(
                        et[:, ts(b, EB), :].bitcast(i16), sps[:],
                        SCH_A, SCH_B, MUL, ADD)
                else:
                    nc.scalar.activation(et[:, ts(b, EB), :], sps[:], EXP)

            def av_pair(qc, qq, k):
                # AV matmuls for key chunks (2k, 2k+1) of quarter (qc,qq)
                et = ets[(qc, qq)]
                if qq == 0 and k == 0:
                    dps = ps_d.tile([128, QCH], f32, tag="dps", name="dps")
                    ops = ps_o.tile([128, QCH], f32, tag="ops", name="ops")
                    state[qc] = [dps, ops]
                ops = state[qc][1]
                for i in (2 * k, 2 * k + 1):
                    kc = qq * KQ + i
                    nc.tensor.matmul(
                        ops[:], V_sb[:, ts(kc, 128)], et[:, i, :],
                        start=(kc == 0), stop=(kc == NKC - 1),
                        skip_group_check=True)

            def denom_grp(qc, qq, h):
                # 4-way column-packed M=32 ones-matmuls (concurrent streams)
                et = ets[(qc, qq)]
                dps = state[qc][0]
                for i in range(4 * h, 4 * h + 4):
                    kc = qq * KQ + i
                    g = kc % 4
                    nc.tensor.matmul(
                        dps[32 * g:32 * (g + 1), :], ones32[:], et[:, i, :],
                        start=(kc < 4), stop=(kc >= NKC - 4),
                        skip_group_check=True, tile_position=(0, 32 * g))
                if h == 1:
                    ets.pop((qc, qq))

            def chunk_close(qc):
                # free the denominator and O accumulator banks right away
                # (short DVE copies; O leaves unnormalized -- 1/d is folded
                # into the final per-half multiplies instead)
                dps, ops = state[qc]
                d_part = wpool.tile([128, QCH], bf16, tag="dpart", name="d_part")
                nc.vector.tensor_copy(d_part[:], dps[:])
                o_sb = wpool.tile([128, QCH], bf16, tag="osb", name="o_sb")
                nc.vector.tensor_copy(o_sb[:], ops[:])
                state[qc] = [d_part, o_sb]

            def tail_stage1(qc):
                # d-fold matmul; 1/d on DVE; out-projection of unnormalized
                # O (depends only on o_sb, so it never waits on the rec)
                d_part, o_sb = state.pop(qc)
                rt = ps_s.tile([128, EB, QCH], f32, tag="sps", name="tl")
                nc.tensor.matmul(rt[:, 0, :], onesR[:], d_part[:])
                rec = wpool.tile([128, QCH], f32, tag="rec", name="rec")
                nc.vector.reciprocal_approx_fast(rec[:], rt[:, 0, :])
                tfr = ps_s.tile([128, EB, QCH], f32, tag="sps", name="tf")
                for t in range(2):
                    nc.tensor.matmul(tfr[:, t, :], ow_sb[:, t, :], o_sb[:])
                tail1[qc] = (rec, tfr)

            def tail_stage2(qc, fine=False):
                # normalize + residual + store (per-half DMAs so the final
                # transfer overlaps the second half's compute; the last
                # chunk uses 256-col pieces so its final, smaller transfer
                # starts as early as possible)
                qsl = ts(qc, QCH)
                rec, tfr = tail1.pop(qc)
                ot = wpool.tile([128, 2, QCH], f32, tag="ost", name="ot")
                pieces = (slice(0, 256), slice(256, 512)) if fine else (
                    slice(0, 512),)
                for t in range(2):
                    for h in pieces:
                        oh = slice(qc * QCH + h.start, qc * QCH + h.stop)
                        nc.vector.tensor_mul(ot[:, t, h], tfr[:, t, h],
                                             rec[:, h])
                        nc.vector.tensor_add(ot[:, t, h], ot[:, t, h],
                                             res_sb[:, t, oh])
                        nc.sync.dma_start(out_r[:, t, oh], ot[:, t, h])

            # ---- the pipeline ----
            # AV/denominator consume quarter J-2 while S/exp produce quarter
            # J: every consumed et tile finished a full quarter earlier, so
            # the in-order PE queue never waits on an exp completion.
            # single dep-free matmul: fills a DMA-wait hole in the PE queue
            # so the HAM busy-window accumulates without resets
            def bridge(n=1):
                wt = ps_s.tile([128, EB, QCH], f32, tag="sps", name="wbr")
                for i in range(min(n, EB)):
                    nc.tensor.matmul(wt[:, i, :], wsrc[:, :128], wsrc[:])

            # ignition: interleave the first K/QT projection halves with the
            # first S batch so the exp stream starts on ~128KB transfers
            HA, HB = slice(0, 256), slice(256, 512)
            k_proj(0, halves=(HA,))
            bridge()
            qt_proj(0, halves=(HA,))
            bridge()
            ets[(0, 0)] = bigpool.tile([128, KQ, QCH], bf16, tag="big", name="et")
            et00 = ets[(0, 0)]
            sps0 = ps_s.tile([128, EB, QCH], f32, tag="sps", name="sps")
            for h in (HA, HB):
                if h is HB:
                    bridge()
                    qt_proj(0, halves=(HB,))
                for i in range(EB):
                    nc.tensor.matmul(sps0[:, i, h], K_sb[:, ts(i, 128)],
                                     QT_sb[:, h])
                nc.scalar.activation(et00[:, 0:EB, h], sps0[:, :, h], EXP)
            bridge()
            k_proj(0, halves=(HB,))

            # bridge the post-ignition DMA wait so the HAM clock-gate never
            # re-throttles before the first full chunk streams
            bridge(2)
            bridge(2)

            jobs = [(qc, qq) for qc in range(NQC) for qq in range(NQUARTER)]
            for j, (qc, qq) in enumerate(jobs):
                prev = jobs[j - 2] if j > 1 else None
                ex = list(extras.get(j, []))
                if j > 0:
                    s_batch(qc, qq, 0)
                if ex:
                    ex.pop(0)()
                if prev:
                    av_pair(*prev, 0)
                s_batch(qc, qq, 1)
                if ex:
                    ex.pop(0)()
                if prev:
                    av_pair(*prev, 1)
                    av_pair(*prev, 2)
                    av_pair(*prev, 3)
                    denom_grp(*prev, 0)
                    denom_grp(*prev, 1)
                s_batch(qc, qq, 2)
                if prev:
                    if prev[1] == NQUARTER - 1:
                        chunk_close(prev[0])
                for e in ex:
                    e()
                s_batch(qc, qq, 3)
                if j == len(jobs) - 1:
                    # catch up: consume the J-1 quarter inside the last
                    # step so the epilogue only holds the final quarter
                    # (after s3, so its exp streams during these matmuls)
                    for k in range(4):
                        av_pair(*jobs[j - 1], k)
                    denom_grp(*jobs[j - 1], 0)
                    denom_grp(*jobs[j - 1], 1)
                if qq == 2 and qc >= 1:
                    tail_stage1(qc - 1)
                    tail_stage2(qc - 1)

            last = jobs[-1]
            av_pair(*last, 0)
            av_pair(*last, 1)
            av_pair(*last, 2)
            denom_grp(*last, 0)
            av_pair(*last, 3)
            denom_grp(*last, 1)
            chunk_close(last[0])
            tail_stage1(last[0])
            tail_stage2(last[0], fine=True)

    nc.compile()
    return nc


def _get_nc():
    if "nc" not in _CACHE:
        _CACHE["nc"] = _build_nc()
    return _CACHE["nc"]


def make_in_maps(rgbd, x, q_w, q_b, k_w, k_b, v_w, v_b, out_w, out_b):
    """Host-side sharding + weight swizzles. Returns per-core input maps."""
    import ml_dtypes

    f = np.float32
    bf = ml_dtypes.bfloat16
    rgbd = np.asarray(rgbd, f)
    x = np.asarray(x, f)
    q_w = np.asarray(q_w, f)
    q_b = np.asarray(q_b, f)
    k_w = np.asarray(k_w, f)
    v_w = np.asarray(v_w, f)
    out_w = np.asarray(out_w, f)
    out_b = np.asarray(out_b, f)
    v_b = np.asarray(v_b, f)

    # [ci_in, co, m] = w[m, co*128 + ci_in]
    def swz(w):
        return np.ascontiguousarray(
            w.reshape(CI, 2, 128).transpose(2, 1, 0).astype(bf))

    qw_sw, kw_sw = swz(q_w), swz(k_w)
    # vwT[cin, co, ci] = v_w[ci, co*128+cin] (moving operand of the V^T mms)
    vwt_sw = np.ascontiguousarray(
        v_w.reshape(CI, 2, 128).transpose(2, 1, 0).astype(bf))
    # ow[ci, t, co] = out_w[t*128+co, ci] (lhsT halves of the out-projection)
    ow_sw = np.ascontiguousarray(
        out_w.reshape(2, 128, CI).transpose(2, 0, 1).astype(bf))
    qb_sw = np.ascontiguousarray(q_b.reshape(CI, 1))

    wpack = np.zeros((128, W_BYTES), np.uint8)
    wpack[:, W_QW:W_QW + 512] = qw_sw.reshape(128, 256).view(np.uint8)
    wpack[:, W_KW:W_KW + 512] = kw_sw.reshape(128, 256).view(np.uint8)
    wpack[:, W_VWT:W_VWT + 512] = vwt_sw.reshape(128, 256).view(np.uint8)
    wpack[:, W_OW:W_OW + 512] = ow_sw.reshape(128, 256).view(np.uint8)
    wpack[:, W_QB:W_QB + 4] = qb_sw.view(np.uint8)

    ob_fused = (out_b + out_w @ v_b).astype(f)            # [C]

    rs_all = rgbd.reshape(B, C, N)
    xs_all = x.reshape(B, C, N)

    in_maps = []
    for core in range(NCORES):
        b, h = divmod(core, 2)
        sl = slice(h * QSH, (h + 1) * QSH)
        in_maps.append(
            {
                "rs": np.ascontiguousarray(rs_all[b].astype(bf)),
                "xs": np.ascontiguousarray(xs_all[b][:, sl].astype(bf)),
                "res": np.ascontiguousarray(
                    (rs_all[b][:, sl] + ob_fused[:, None]).astype(bf)),
                "wp": wpack,
            }
        )
    return in_maps


def gather_out(results):
    out = np.empty((B, C, N), np.float32)
    for core in range(NCORES):
        b, h = divmod(core, 2)
        out[b][:, h * QSH : (h + 1) * QSH] = results[core]["out"]
    return out.reshape(B, C, HH, WW)


def kernel(**inputs):
    from concourse.bass_utils import run_bass_kernel_spmd

    in_maps = make_in_maps(**inputs)
    nc = _get_nc()
    res = run_bass_kernel_spmd(nc, in_maps, list(range(NCORES)))
    return gather_out(res.results)


# revision 47
# speedup vs baseline: 1.0013x; 1.0013x over previous
"""Cross-attention kernel for Trainium2 (8 NeuronCores, Bass/Tile).

Problem: nn_CrossAttention — B=4, C=256, H=W=64 (N=4096 tokens), CI=128.
  q = q_w @ x + q_b            [B, N, CI]
  k = k_w @ rgbd + k_b         [B, CI, N]
  v = v_w @ rgbd + v_b         [B, N, CI]
  out = rgbd + out_w @ (softmax(q k) v) + out_b

Sharding: data-parallel over batch x query-half. Core i handles batch i//2,
query half i%2 (2048 queries, all 4096 keys). No collectives needed.

Math simplifications (exact):
  - k_b drops out of softmax (adds a per-query constant to logits).
  - v_b commutes with the softmax average; out_b + out_w @ v_b is folded
    into the residual on the host (res' = rgbd_slice + fused bias).
  - exp() without max-subtraction: logits are bounded (|S| <~ 45), safe fp32.

Engine plan (per core) — PE is the wall (~86us of matmul streaming), so the
kernel is one long software pipeline in which the in-order PE queue never
waits on anything:
  - One packed weights DMA (HWDGE dispatches cost ~650ns each on the serial
    Sync queue, and only ~8 transfers fit the DMA ring window); rs-heavy
    dispatch interleave since K/V^T consume rs long before QT needs xs.
  - Dep-free warmup matmuls bridge the input-DMA wait so the PE HAM
    clock-gate is at 2.4GHz when real work lands; an "ignition" first
    S-batch runs in 256-query halves off ~128KB first transfers.
  - V^T[k, ci] is computed directly as 64 small matmuls with rs chunks as
    the stationary operand (no PE transposes; ~56ns/MM issue cadence).
  - exp is split across engines: ScalarE runs true EXP ACTIVATEs; DVE runs
    Schraudolph bit-trick exp (x*128*log2e + 128*127-5 -> int16, bitcast
    bf16, ~3% rel err) for batches b3 (+b2 in tail-free steps).  Neither
    engine saturates, and the 6-bank S-ring (3 tiles x [128,2,512]) plus
    J-2 consumption lag keep every PE operand ready a quarter in advance:
    steady-state periods are set purely by PE matmul throughput.
  - Denominator rows accumulate via 4-way column-packed M=32 ones-matmuls
    (4 concurrent col-group streams); a bf16 ones/32 matmul folds them to
    d broadcast across partitions.
  - 1/d via DVE reciprocal_approx_fast (no ScalarE Ln -> exactly one ACT
    table load for the whole kernel).  O leaves PSUM unnormalized at chunk
    close (frees the single accumulator bank early); the out-projection
    runs on unnormalized O and 1/d folds into the final per-half multiply.
  - Tails (fold, rec, out-proj, normalize+residual, per-half store) run in
    stages inside later steps' slack; the last step catches up the J-1
    quarter after its final S batch so the epilogue is one quarter deep.
"""

import numpy as np

B, C, HH, WW = 4, 256, 64, 64
CI = 128
N = HH * WW            # 4096 tokens per batch
NCORES = 8
QSH = N // 2           # 2048 queries per core
QCH = 512              # query chunk (matmul moving free dim)
NQC = QSH // QCH       # 4 q-chunks
NKC = N // 128         # 32 key chunks of 128
EB = 2                 # S tiles per exp batch (PSUM-sourced ACTIVATE)
KQ = 8                 # key chunks per ET quarter-buffer
NQUARTER = NKC // KQ   # 4 quarters

# packed-weights byte offsets (per partition)
W_QW, W_KW, W_VWT, W_OW, W_QB = 0, 512, 1024, 1536, 2048
W_BYTES = 2052

_CACHE = {}


def _build_nc():
    import concourse.bass as bass
    import concourse.mybir as mybir
    import concourse.tile as tile
    from concourse import bacc
    from concourse.bass import ts

    f32 = mybir.dt.float32
    f32r = mybir.dt.float32r
    bf16 = mybir.dt.bfloat16
    u8 = mybir.dt.uint8
    EXP = mybir.ActivationFunctionType.Exp

    nc = bacc.Bacc("TRN2", target_bir_lowering=False, debug=False)

    rs_d = nc.dram_tensor("rs", [C, N], bf16, kind="ExternalInput")
    xs_d = nc.dram_tensor("xs", [C, QSH], bf16, kind="ExternalInput")
    res_d = nc.dram_tensor("res", [C, QSH], bf16, kind="ExternalInput")
    wp_d = nc.dram_tensor("wp", [128, W_BYTES], u8, kind="ExternalInput")
    out_d = nc.dram_tensor("out", [C, QSH], f32, kind="ExternalOutput")

    rs_r = rs_d.ap().rearrange("(co ci) n -> ci co n", ci=128)
    xs_r = xs_d.ap().rearrange("(co ci) n -> ci co n", ci=128)
    res_r = res_d.ap().rearrange("(co ci) n -> ci co n", ci=128)
    out_r = out_d.ap().rearrange("(co ci) n -> ci co n", ci=128)

    with tile.TileContext(nc) as tc:
        with (
            tc.tile_pool(name="const", bufs=1) as cpool,
            tc.tile_pool(name="big", bufs=3) as bigpool,
            tc.tile_pool(name="work", bufs=2) as wpool,
            tc.tile_pool(name="ps_s", bufs=3, space=bass.MemorySpace.PSUM) as ps_s,
            tc.tile_pool(name="ps_d", bufs=1, space=bass.MemorySpace.PSUM) as ps_d,
            tc.tile_pool(name="ps_o", bufs=1, space=bass.MemorySpace.PSUM) as ps_o,
        ):
            # ---- one packed weight DMA, then activations (xs before rs) ----
            wp_sb = cpool.tile([128, W_BYTES], u8, tag="wp")
            nc.sync.dma_start(wp_sb[:], wp_d.ap())

            rs_sb = cpool.tile([128, 2, N], bf16, tag="rs")
            xs_sb = cpool.tile([128, 2, QSH], bf16, tag="xs")
            res_sb = cpool.tile([128, 2, QSH], bf16, tag="res")
            # small first chunks so the first S batch (and its exp) ignites
            # on ~128KB transfers; then rs-heavy interleave -- the K/V^T
            # projections consume rs chunks much earlier than QT needs xs,
            # and only ~8 transfers fit in the DMA ring window at once
            nc.sync.dma_start(rs_sb[:, :, 0:256], rs_r[:, :, 0:256])
            nc.sync.dma_start(xs_sb[:, :, 0:256], xs_r[:, :, 0:256])
            nc.sync.dma_start(xs_sb[:, :, 256:512], xs_r[:, :, 256:512])
            nc.sync.dma_start(rs_sb[:, :, 256:512], rs_r[:, :, 256:512])
            xs_order = [None, 1, None, 2, None, None, 3, None]
            for j in range(1, 8):
                nc.sync.dma_start(rs_sb[:, :, ts(j, 512)], rs_r[:, :, ts(j, 512)])
                xj = xs_order[j]
                if xj is not None:
                    nc.sync.dma_start(xs_sb[:, :, ts(xj, 512)],
                                      xs_r[:, :, ts(xj, 512)])
            for j in range(NQC):
                nc.sync.dma_start(res_sb[:, :, ts(j, QCH)], res_r[:, :, ts(j, QCH)])

            qw_sb = wp_sb[:, W_QW:W_QW + 512].bitcast(bf16).rearrange(
                "p (co m) -> p co m", co=2)
            kw_sb = wp_sb[:, W_KW:W_KW + 512].bitcast(bf16).rearrange(
                "p (co m) -> p co m", co=2)
            vwt_sb = wp_sb[:, W_VWT:W_VWT + 512].bitcast(bf16).rearrange(
                "p (co m) -> p co m", co=2)
            ow_sb = wp_sb[:, W_OW:W_OW + 512].bitcast(bf16).rearrange(
                "p (t m) -> p t m", t=2)
            qb_sb = wp_sb[:, W_QB:W_QB + 4].bitcast(f32)

            # ---- constants (wsrc first: it gates the warmup matmuls) ----
            wsrc = cpool.tile([128, 512], bf16, tag="wsrc")
            nc.vector.memset(wsrc[:], 1.0)
            ones_f = cpool.tile([128, 128], f32, tag="ones_f")
            nc.vector.memset(ones_f[:], 1.0)
            ones32 = cpool.tile([128, 32], bf16, tag="ones32")
            nc.vector.tensor_copy(ones32[:], ones_f[:, :32])
            onesR = cpool.tile([128, 128], bf16, tag="onesR")
            nc.vector.memset(onesR[:], 1.0 / 32.0)

            # warm the ScalarE exp table while input DMAs run
            warm = cpool.tile([128, 1], f32, tag="warm")
            nc.scalar.activation(warm[:], ones_f[:, :1], EXP)

            # dep-free dummy matmuls: trip the PE HAM activity window during
            # the input-DMA wait so real work runs at full clock (they live
            # in the ps_s ring)
            for w in range(3):
                wps = ps_s.tile([128, EB, QCH], f32, tag="sps", name="wps")
                for i in range(EB):
                    nc.tensor.matmul(wps[:, i, :], wsrc[:, :128], wsrc[:])

            K_sb = cpool.tile([128, N], bf16, tag="K")
            QT_sb = cpool.tile([128, QSH], bf16, tag="QT")
            V_sb = cpool.tile([128, N], bf16, tag="V")

            # ---- prologue PE jobs (emitted into early pipeline slack;
            # transient psum comes from the S ring -- bufs=3 absorbs it) ----
            def qt_proj(p, halves=(slice(0, 512),)):
                rt = ps_s.tile([128, EB, QCH], f32, tag="sps", name="qtps")
                ps = rt[:, 0, :]
                base = 512 * p
                for h in halves:
                    for co in range(2):
                        nc.tensor.matmul(
                            ps[:, h], qw_sb[:, co, :],
                            xs_sb[:, co, base + h.start:base + h.stop],
                            start=(co == 0), stop=(co == 1))
                    nc.vector.tensor_scalar_add(
                        QT_sb[:, base + h.start:base + h.stop], ps[:, h], qb_sb)

            def k_proj(p, halves=(slice(0, 512),)):
                rt = ps_s.tile([128, EB, QCH], f32, tag="sps", name="kps")
                ps = rt[:, 0, :]
                base = 512 * p
                for h in halves:
                    for co in range(2):
                        nc.tensor.matmul(
                            ps[:, h], kw_sb[:, co, :],
                            rs_sb[:, co, base + h.start:base + h.stop],
                            start=(co == 0), stop=(co == 1))
                    nc.vector.tensor_copy(
                        K_sb[:, base + h.start:base + h.stop], ps[:, h])

            def vt_group(g):
                # V^T[k, ci] for key chunks 4g..4g+3: rs chunk stationary,
                # v_wT moving -> output keys on partitions (no transposes)
                rt = ps_s.tile([128, EB, QCH], f32, tag="sps", name="vps")
                ps = rt[:, 0, :]
                for j in range(4):
                    kc = 4 * g + j
                    for co in range(2):
                        nc.tensor.matmul(
                            ps[:, ts(j, 128)],
                            rs_sb[:, co, ts(kc, 128)],
                            vwt_sb[:, co, :],
                            start=(co == 0), stop=(co == 1))
                nc.vector.tensor_copy(V_sb[:, ts(g, 512)], ps)

            # extras[j] = prologue jobs to interleave into quarter-step j
            extras = {
                0: [lambda: k_proj(1), lambda: vt_group(0), lambda: k_proj(2),
                    lambda: vt_group(1), lambda: k_proj(3)],
                1: [lambda: k_proj(4), lambda: vt_group(2), lambda: vt_group(3)],
                2: [lambda: k_proj(5), lambda: vt_group(4), lambda: k_proj(6),
                    lambda: vt_group(5)],
                3: [lambda: k_proj(7), lambda: vt_group(6), lambda: vt_group(7),
                    lambda: qt_proj(1)],
                6: [lambda: qt_proj(2)],
                10: [lambda: qt_proj(3)],
            }

            # ---- pipeline state ----
            state = {}     # qc -> [dps, ops] PSUM accumulators
            ets = {}       # (qc,qq) -> et tile
            tail1 = {}     # qc -> (d_part, rec, o_sc) after stage 1

            # exp(x) ~= bitcast_bf16(int16(x*128*log2(e) + 128*127 - c)):
            # Schraudolph's approximation (~3% rel err; fine at 2e-2 tol).
            # Batches b2/b3 of each quarter run on DVE this way: the psum
            # ring slots that gate the next quarter's first S matmuls are
            # then freed by the (early, slack) DVE queue instead of by
            # ScalarE's last in-order exp -- PE never stalls on the ring.
            SCH_A = 128.0 / float(np.log(2.0))
            SCH_B = 128.0 * 127.0 - 5.0
            i16 = mybir.dt.int16
            MUL, ADD = mybir.AluOpType.mult, mybir.AluOpType.add

            def s_batch(qc, qq, b):
                # two S matmuls + one EXP covering key chunks (2b, 2b+1)
                qsl = ts(qc, QCH)
                if (qc, qq) not in ets:
                    ets[(qc, qq)] = bigpool.tile([128, KQ, QCH], bf16, tag="big", name="et")
                et = ets[(qc, qq)]
                sps = ps_s.tile([128, EB, QCH], f32, tag="sps", name="sps")
                for i in range(EB):
                    kc = qq * KQ + b * EB + i
                    nc.tensor.matmul(sps[:, i, :], K_sb[:, ts(kc, 128)],
                                     QT_sb[:, qsl])
                # DVE takes b3 (and b2 in steps without tail DVE work) so
                # neither exp engine saturates.
                on_dve = (b == 3 and not (qc == 3 and qq == 3)) or (
                    b == 2 and qq in (0, 1))
                if on_dve:
                    nc.vector.tensor_scalar(
                        et[:, ts(b, EB), :].bitcast(i16), sps[:],
                        SCH_A, SCH_B, MUL, ADD)
                else:
                    nc.scalar.activation(et[:, ts(b, EB), :], sps[:], EXP)

            def av_pair(qc, qq, k):
                # AV matmuls for key chunks (2k, 2k+1) of quarter (qc,qq)
                et = ets[(qc, qq)]
                if qq == 0 and k == 0:
                    dps = ps_d.tile([128, QCH], f32, tag="dps", name="dps")
                    ops = ps_o.tile([128, QCH], f32, tag="ops", name="ops")
                    state[qc] = [dps, ops]
                ops = state[qc][1]
                for i in (2 * k, 2 * k + 1):
                    kc = qq * KQ + i
                    nc.tensor.matmul(
                        ops[:], V_sb[:, ts(kc, 128)], et[:, i, :],
                        start=(kc == 0), stop=(kc == NKC - 1),
                        skip_group_check=True)

            def denom_grp(qc, qq, h):
                # 4-way column-packed M=32 ones-matmuls (concurrent streams)
                et = ets[(qc, qq)]
                dps = state[qc][0]
                for i in range(4 * h, 4 * h + 4):
                    kc = qq * KQ + i
                    g = kc % 4
                    nc.tensor.matmul(
                        dps[32 * g:32 * (g + 1), :], ones32[:], et[:, i, :],
                        start=(kc < 4), stop=(kc >= NKC - 4),
                        skip_group_check=True, tile_position=(0, 32 * g))
                if h == 1:
                    ets.pop((qc, qq))

            def chunk_close(qc):
                # free the denominator and O accumulator banks right away
                # (short DVE copies; O leaves unnormalized -- 1/d is folded
                # into the final per-half multiplies instead)
                dps, ops = state[qc]
                d_part = wpool.tile([128, QCH], bf16, tag="dpart", name="d_part")
                nc.vector.tensor_copy(d_part[:], dps[:])
                o_sb = wpool.tile([128, QCH], bf16, tag="osb", name="o_sb")
                nc.vector.tensor_copy(o_sb[:], ops[:])
                state[qc] = [d_part, o_sb]

            def tail_stage1(qc):
                # d-fold matmul; 1/d on DVE; out-projection of unnormalized
                # O (depends only on o_sb, so it never waits on the rec)
                d_part, o_sb = state.pop(qc)
                rt = ps_s.tile([128, EB, QCH], f32, tag="sps", name="tl")
                nc.tensor.matmul(rt[:, 0, :], onesR[:], d_part[:])
                rec = wpool.tile([128, QCH], f32, tag="rec", name="rec")
                nc.vector.reciprocal_approx_fast(rec[:], rt[:, 0, :])
                tfr = ps_s.tile([128, EB, QCH], f32, tag="sps", name="tf")
                for t in range(2):
                    nc.tensor.matmul(tfr[:, t, :], ow_sb[:, t, :], o_sb[:])
                tail1[qc] = (rec, tfr)

            def tail_stage2(qc, fine=False):
                # normalize + residual + store (per-half DMAs so the final
                # transfer overlaps the second half's compute; the last
                # chunk uses 256-col pieces so its final, smaller transfer
                # starts as early as possible)
                qsl = ts(qc, QCH)
                rec, tfr = tail1.pop(qc)
                ot = wpool.tile([128, 2, QCH], f32, tag="ost", name="ot")
                pieces = (slice(0, 256), slice(256, 512)) if fine else (
                    slice(0, 512),)
                for t in range(2):
                    for h in pieces:
                        oh = slice(qc * QCH + h.start, qc * QCH + h.stop)
                        nc.vector.tensor_mul(ot[:, t, h], tfr[:, t, h],
                                             rec[:, h])
                        nc.vector.tensor_add(ot[:, t, h], ot[:, t, h],
                                             res_sb[:, t, oh])
                        nc.sync.dma_start(out_r[:, t, oh], ot[:, t, h])

            # ---- the pipeline ----
            # AV/denominator consume quarter J-2 while S/exp produce quarter
            # J: every consumed et tile finished a full quarter earlier, so
            # the in-order PE queue never waits on an exp completion.
            # single dep-free matmul: fills a DMA-wait hole in the PE queue
            # so the HAM busy-window accumulates without resets
            def bridge(n=1):
                wt = ps_s.tile([128, EB, QCH], f32, tag="sps", name="wbr")
                for i in range(min(n, EB)):
                    nc.tensor.matmul(wt[:, i, :], wsrc[:, :128], wsrc[:])

            # ignition: interleave the first K/QT projection halves with the
            # first S batch so the exp stream starts on ~128KB transfers
            HA, HB = slice(0, 256), slice(256, 512)
            k_proj(0, halves=(HA,))
            bridge()
            qt_proj(0, halves=(HA,))
            bridge()
            ets[(0, 0)] = bigpool.tile([128, KQ, QCH], bf16, tag="big", name="et")
            et00 = ets[(0, 0)]
            sps0 = ps_s.tile([128, EB, QCH], f32, tag="sps", name="sps")
            for h in (HA, HB):
                if h is HB:
                    bridge()
                    qt_proj(0, halves=(HB,))
                for i in range(EB):
                    nc.tensor.matmul(sps0[:, i, h], K_sb[:, ts(i, 128)],
                                     QT_sb[:, h])
                nc.scalar.activation(et00[:, 0:EB, h], sps0[:, :, h], EXP)
            bridge()
            k_proj(0, halves=(HB,))

            # bridge the post-ignition DMA wait so the HAM clock-gate never
            # re-throttles before the first full chunk streams
            bridge(2)
            bridge(2)

            jobs = [(qc, qq) for qc in range(NQC) for qq in range(NQUARTER)]
            for j, (qc, qq) in enumerate(jobs):
                prev = jobs[j - 2] if j > 1 else None
                ex = list(extras.get(j, []))
                if j > 0:
                    s_batch(qc, qq, 0)
                if ex:
                    ex.pop(0)()
                if prev:
                    av_pair(*prev, 0)
                s_batch(qc, qq, 1)
                if ex:
                    ex.pop(0)()
                if prev:
                    av_pair(*prev, 1)
                    av_pair(*prev, 2)
                    av_pair(*prev, 3)
                    denom_grp(*prev, 0)
                    denom_grp(*prev, 1)
                s_batch(qc, qq, 2)
                if prev:
                    if prev[1] == NQUARTER - 1:
                        chunk_close(prev[0])
                for e in ex:
                    e()
                s_batch(qc, qq, 3)
                if j == len(jobs) - 1:
                    # catch up: consume the J-1 quarter inside the last
                    # step so the epilogue only holds the final quarter
                    # (after s3, so its exp streams during these matmuls)
                    for k in range(4):
                        av_pair(*jobs[j - 1], k)
                    denom_grp(*jobs[j - 1], 0)
                    denom_grp(*jobs[j - 1], 1)
                if qq == 2 and qc >= 1:
                    tail_stage1(qc - 1)
                    tail_stage2(qc - 1)

            last = jobs[-1]
            av_pair(*last, 0)
            av_pair(*last, 1)
            av_pair(*last, 2)
            denom_grp(*last, 0)
            av_pair(*last, 3)
            denom_grp(*last, 1)
            chunk_close(last[0])
            tail_stage1(last[0])
            tail_stage2(last[0], fine=True)

    nc.compile()
    return nc


def _get_nc():
    if "nc" not in _CACHE:
        _CACHE["nc"] = _build_nc()
    return _CACHE["nc"]


def make_in_maps(rgbd, x, q_w, q_b, k_w, k_b, v_w, v_b, out_w, out_b):
    """Host-side sharding + weight swizzles. Returns per-core input maps."""
    import ml_dtypes

    f = np.float32
    bf = ml_dtypes.bfloat16
    rgbd = np.asarray(rgbd, f)
    x = np.asarray(x, f)
    q_w = np.asarray(q_w, f)
    q_b = np.asarray(q_b, f)
    k_w = np.asarray(k_w, f)
    v_w = np.asarray(v_w, f)
    out_w = np.asarray(out_w, f)
    out_b = np.asarray(out_b, f)
    v_b = np.asarray(v_b, f)

    # [ci_in, co, m] = w[m, co*128 + ci_in]
    def swz(w):
        return np.ascontiguousarray(
            w.reshape(CI, 2, 128).transpose(2, 1, 0).astype(bf))

    qw_sw, kw_sw = swz(q_w), swz(k_w)
    # vwT[cin, co, ci] = v_w[ci, co*128+cin] (moving operand of the V^T mms)
    vwt_sw = np.ascontiguousarray(
        v_w.reshape(CI, 2, 128).transpose(2, 1, 0).astype(bf))
    # ow[ci, t, co] = out_w[t*128+co, ci] (lhsT halves of the out-projection)
    ow_sw = np.ascontiguousarray(
        out_w.reshape(2, 128, CI).transpose(2, 0, 1).astype(bf))
    qb_sw = np.ascontiguousarray(q_b.reshape(CI, 1))

    wpack = np.zeros((128, W_BYTES), np.uint8)
    wpack[:, W_QW:W_QW + 512] = qw_sw.reshape(128, 256).view(np.uint8)
    wpack[:, W_KW:W_KW + 512] = kw_sw.reshape(128, 256).view(np.uint8)
    wpack[:, W_VWT:W_VWT + 512] = vwt_sw.reshape(128, 256).view(np.uint8)
    wpack[:, W_OW:W_OW + 512] = ow_sw.reshape(128, 256).view(np.uint8)
    wpack[:, W_QB:W_QB + 4] = qb_sw.view(np.uint8)

    ob_fused = (out_b + out_w @ v_b).astype(f)            # [C]

    rs_all = rgbd.reshape(B, C, N)
    xs_all = x.reshape(B, C, N)

    in_maps = []
    for core in range(NCORES):
        b, h = divmod(core, 2)
        sl = slice(h * QSH, (h + 1) * QSH)
        in_maps.append(
            {
                "rs": np.ascontiguousarray(rs_all[b].astype(bf)),
                "xs": np.ascontiguousarray(xs_all[b][:, sl].astype(bf)),
                "res": np.ascontiguousarray(
                    (rs_all[b][:, sl] + ob_fused[:, None]).astype(bf)),
                "wp": wpack,
            }
        )
    return in_maps


def gather_out(results):
    out = np.empty((B, C, N), np.float32)
    for core in range(NCORES):
        b, h = divmod(core, 2)
        out[b][:, h * QSH : (h + 1) * QSH] = results[core]["out"]
    return out.reshape(B, C, HH, WW)


def kernel(**inputs):
    from concourse.bass_utils import run_bass_kernel_spmd

    in_maps = make_in_maps(**inputs)
    nc = _get_nc()
    res = run_bass_kernel_spmd(nc, in_maps, list(range(NCORES)))
    return gather_out(res.results)
